# revision 2
# baseline (speedup 1.0000x reference)
"""AWQ int4 linear kernel for Trainium2 (8 NeuronCores, SPMD).

Computes: out = (x * input_scale) @ dequant(qweight, scales, zeros).T + bias

  x:           [4, 2048, 4096] f32
  qweight:     [11008, 2048]   i32  (byte values 0..255; two 4-bit codes each,
                                     high nibble first -> in-position 2j, low -> 2j+1)
  scales/zeros:[11008, 32]     f32  (per 128-wide input group)
  input_scale: [4096]          f32
  bias:        [11008]         f32
  out:         [4, 2048, 11008] f32

Sharding: 4-way over tokens x 2-way over out_features (core = r*2 + c).
Per core: M=2048 tokens, K=4096, N=5504 outs.

Per-core kernel (v6 -- "PE does nothing but matmuls"):
  - All transposes go through the DMA XBAR (dma_start_transpose, f16), not
    the PE: x [t,k]->[k,t], dequantized weights [o,k]->[k,o], and the output
    [o,t]->[t,o].  PE cycles = pure matmul (2048*4096*5504/128^2 = 2.82M).
  - x path: DMA f32 chunk -> DVE multiply by a partition-replicated
    input_scale row (converts to f16) -> XBAR into the resident xsT
    [k, t] tile (128 KiB/partition).
  - W path (per 128-wide out-feature block): DMA packed qweight, DVE nibble
    unpack (shift/and), fused per-group dequant (q - zero) * scale i32->f16,
    XBAR into W [k, o-block].
  - Matmuls: stationary W[k,128o] reused over 4 moving xsT[k,512t] tiles
    (amortizes LDWEIGHTS 4x), accumulate 32 k-tiles in PSUM [o,t] f32.
  - Drain: DVE adds bias (per-partition = out feature) f32->f16, XBAR
    transposes [o,t]->[t,o], SWDGE (gpsimd) DMA casts f16->f32 into HBM.
"""

import os
import sys

for _p in ("/opt/trn_rl_repo",):
    if _p not in sys.path and os.path.isdir(_p):
        sys.path.insert(0, _p)

import numpy as np

import concourse.bass as bass
import concourse.mybir as mybir
from concourse import bacc
from concourse.tile import TileContext

F32 = mybir.dt.float32
F16 = mybir.dt.float16
I32 = mybir.dt.int32

# Full problem shape
T_FULL = 8192
K_FULL = 4096
O_FULL = 11008

# Sharding: R-way over tokens, C-way over out_features
R_SHARDS = 4
C_SHARDS = 2
N_CORES = 8
KERNEL_REV = 6   # bump on every kernel change (feeds the fingerprint tag)


def build_nc(T, K, O, loop_n=1):
    """Build the per-core Bass program. T tokens, K in-features, O out-features."""
    assert T % 128 == 0 and K % 512 == 0 and O % 128 == 0
    KT = K // 128          # k-tiles == dequant groups (group size 128)
    TT = T // 128
    OS = O // 128          # out-feature subtiles (one stationary block each)
    TGW = min(512, T)      # moving-operand width (tokens) per matmul
    TG = T // TGW
    KP = min(1024, K)      # k-piece for unpack/dequant/xbar staging
    NPIECE = K // KP
    XKG = min(512, K)      # x DMA chunk width along k

    nc = bacc.Bacc()

    x_d = nc.dram_tensor("x", [T, K], F32, kind="ExternalInput")
    qw_d = nc.dram_tensor("qweight", [O, K // 2], I32, kind="ExternalInput")
    sc_d = nc.dram_tensor("scales", [O, KT], F32, kind="ExternalInput")
    zr_d = nc.dram_tensor("zeros", [O, KT], F32, kind="ExternalInput")
    isc_d = nc.dram_tensor("input_scale", [K], F32, kind="ExternalInput")
    b_d = nc.dram_tensor("bias", [O], F32, kind="ExternalInput")
    out_d = nc.dram_tensor("out", [T, O], F32, kind="ExternalOutput")
    # shape-bearing version tag: makes each build's HLO fingerprint unique so
    # the XLA/neuron compile caches can never alias two different BIRs
    tag_d = nc.dram_tensor("bench_tag", [1, KERNEL_REV * 16 + loop_n], F32,
                           kind="ExternalInput")

    with TileContext(nc) as tc:
        with tc.tile_pool(name="persist", bufs=1) as persist:
            # xsT: resident transposed/scaled activations, f16, kt-major
            xsT = persist.tile([128, KT * T], F16, tag="xsT")
            # input_scale replicated to all partitions (for the pre-transpose
            # scale+convert in [t, k] layout, where k is the free dim)
            isc_rep = persist.tile([128, K], F32, tag="iscrep")
            bias_sb = persist.tile([128, OS], F32, tag="bias")
            nc.sync.dma_start(out=bias_sb, in_=b_d.rearrange("(a b) -> b a", b=128))
            nc.sync.dma_start(out=isc_rep[0:1, :],
                              in_=isc_d.rearrange("(one a) -> one a", one=1))
            nc.gpsimd.partition_broadcast(isc_rep[:, :], isc_rep[0:1, :])
            tag_sb = persist.tile([1, KERNEL_REV * 16 + loop_n], F32, tag="tag")
            nc.sync.dma_start(out=tag_sb, in_=tag_d[:])

            import contextlib
            loop_cm = tc.For_i(0, loop_n, 1) if loop_n > 1 else contextlib.nullcontext()
            with loop_cm:
                with (
                    tc.tile_pool(name="xin", bufs=3) as xin_pool,
                    tc.tile_pool(name="xb", bufs=3) as xb_pool,
                    tc.tile_pool(name="qw", bufs=3) as qw_pool,
                    tc.tile_pool(name="qi", bufs=2) as qi_pool,
                    tc.tile_pool(name="qd", bufs=2) as qd_pool,
                    tc.tile_pool(name="wt", bufs=3) as w_pool,
                    tc.tile_pool(name="sz", bufs=2) as sz_pool,
                    tc.tile_pool(name="osb", bufs=3) as osb_pool,
                    tc.tile_pool(name="otb", bufs=3) as otb_pool,
                    tc.tile_pool(name="pso", bufs=8, space="PSUM") as pso_pool,
                ):
                    # --- prologue: x -> xsT via scale+convert + XBAR ---
                    for tb in range(TT):
                        for kg in range(K // XKG):
                            xin = xin_pool.tile([128, XKG], F32, tag="xin")
                            nc.sync.dma_start(
                                out=xin,
                                in_=x_d[tb*128:(tb+1)*128, kg*XKG:(kg+1)*XKG])
                            xb = xb_pool.tile([128, XKG], F16, tag="xb")
                            nc.vector.tensor_tensor(
                                out=xb, in0=xin,
                                in1=isc_rep[:, kg*XKG:(kg+1)*XKG],
                                op=mybir.AluOpType.mult)
                            nj = XKG // 128
                            dst = xsT.rearrange("p (j t) -> p j t", t=T)[
                                :, kg*nj:(kg+1)*nj, tb*128:(tb+1)*128]
                            nc.scalar.dma_start_transpose(dst, xb)

                    # --- main: per 128-wide out-feature block ---
                    for osi in range(OS):
                        sc_t = sz_pool.tile([128, KT], F32, tag="sc")
                        zr_t = sz_pool.tile([128, KT], F32, tag="zr")
                        nc.sync.dma_start(out=sc_t, in_=sc_d[osi*128:(osi+1)*128, :])
                        nc.sync.dma_start(out=zr_t, in_=zr_d[osi*128:(osi+1)*128, :])
                        w_t = w_pool.tile([128, KT * 128], F16, tag="wt")
                        for pc in range(NPIECE):
                            qw_t = qw_pool.tile([128, KP // 2], I32, tag="qw")
                            nc.sync.dma_start(
                                out=qw_t,
                                in_=qw_d[osi*128:(osi+1)*128,
                                         pc*(KP//2):(pc+1)*(KP//2)])
                            # unpack to int32 (bit ops can't cast dtypes):
                            # high nibble -> even k, low nibble -> odd k
                            qi = qi_pool.tile([128, KP], I32, tag="qi")
                            nc.vector.tensor_scalar(
                                qi[:, ::2], qw_t, 4, None,
                                op0=mybir.AluOpType.logical_shift_right)
                            nc.vector.tensor_scalar(
                                qi[:, 1::2], qw_t, 15, None,
                                op0=mybir.AluOpType.bitwise_and)
                            # per-group dequant: (q - zero) * scale, i32 -> f16
                            qd = qd_pool.tile([128, KP], F16, tag="qd")
                            for gl in range(KP // 128):
                                g = pc * (KP // 128) + gl
                                nc.vector.tensor_scalar(
                                    qd[:, gl*128:(gl+1)*128],
                                    qi[:, gl*128:(gl+1)*128],
                                    zr_t[:, g:g+1], sc_t[:, g:g+1],
                                    op0=mybir.AluOpType.subtract,
                                    op1=mybir.AluOpType.mult)
                            # XBAR [o,k] -> [k,o]: dest is contiguous because
                            # W is kt-major with 128-wide o blocks
                            dst = w_t[:, pc*KP:(pc+1)*KP].rearrange(
                                "p (j c) -> p j c", c=128)
                            nc.scalar.dma_start_transpose(dst, qd)

                        # matmuls: psum[o,t] += W[k,o-128].T @ xsT[k,t-512];
                        # stationary W reused across the TG token groups.
                        ps = [pso_pool.tile([128, TGW], F32, tag="pso",
                                            name=f"pso{osi}_{tg}")
                              for tg in range(TG)]
                        for kt in range(KT):
                            lhsT = w_t[:, kt*128:(kt+1)*128]
                            for tg in range(TG):
                                nc.tensor.matmul(
                                    ps[tg], lhsT,
                                    xsT[:, kt*T + tg*TGW : kt*T + (tg+1)*TGW],
                                    start=(kt == 0), stop=(kt == KT - 1))
                        for tg in range(TG):
                            # bias add (per-partition = out feature) fused with
                            # the PSUM->SBUF drain, f32 -> f16
                            osb = osb_pool.tile([128, TGW], F16, tag="osb")
                            nc.vector.tensor_scalar(
                                osb, ps[tg], bias_sb[:, osi:osi+1], None,
                                op0=mybir.AluOpType.add)
                            # XBAR [o,t] -> [t,o], then SWDGE DMA casts f16->f32
                            otb = otb_pool.tile([128, TGW], F16, tag="otb")
                            nc.scalar.dma_start_transpose(
                                otb.rearrange("p (j c) -> p j c", c=128), osb)
                            dst = out_d[tg*TGW:(tg+1)*TGW,
                                        osi*128:(osi+1)*128].rearrange(
                                "(j p) c -> p j c", p=128)
                            nc.gpsimd.dma_start(
                                out=dst,
                                in_=otb.rearrange("p (j c) -> p j c", c=128))
    nc.finalize()
    return nc


_CACHED = {}


def _get_nc(T, K, O):
    key = (T, K, O)
    if key not in _CACHED:
        _CACHED[key] = build_nc(T, K, O)
    return _CACHED[key]


LAST_RESULT = {}


def make_in_maps(x, qweight, scales, zeros, input_scale, bias):
    """Shard the full inputs into per-core input maps."""
    x = np.ascontiguousarray(np.asarray(x, dtype=np.float32)).reshape(T_FULL, K_FULL)
    qweight = np.ascontiguousarray(np.asarray(qweight, dtype=np.int32))
    scales = np.ascontiguousarray(np.asarray(scales, dtype=np.float32))
    zeros = np.ascontiguousarray(np.asarray(zeros, dtype=np.float32))
    input_scale = np.ascontiguousarray(np.asarray(input_scale, dtype=np.float32))
    bias = np.ascontiguousarray(np.asarray(bias, dtype=np.float32))

    T = T_FULL // R_SHARDS
    O = O_FULL // C_SHARDS
    in_maps = []
    for core in range(N_CORES):
        r, c = core // C_SHARDS, core % C_SHARDS
        in_maps.append({
            "x": x[r * T:(r + 1) * T],
            "qweight": qweight[c * O:(c + 1) * O],
            "scales": scales[c * O:(c + 1) * O],
            "zeros": zeros[c * O:(c + 1) * O],
            "input_scale": input_scale,
            "bias": bias[c * O:(c + 1) * O],
            "bench_tag": np.zeros((1, KERNEL_REV * 16 + 1), dtype=np.float32),
        })
    return in_maps


def kernel(x, qweight, scales, zeros, input_scale, bias):
    from concourse.bass_utils import run_bass_kernel_spmd

    T = T_FULL // R_SHARDS
    O = O_FULL // C_SHARDS

    nc = _get_nc(T, K_FULL, O)
    in_maps = make_in_maps(x, qweight, scales, zeros, input_scale, bias)

    res = run_bass_kernel_spmd(
        nc, in_maps, list(range(N_CORES)),
        trace=bool(os.environ.get("AWQ_TRACE")),
    )
    LAST_RESULT["exec_time_ns"] = res.exec_time_ns
    LAST_RESULT["profile_json"] = res.profile_json

    out = np.empty((T_FULL, O_FULL), dtype=np.float32)
    for core in range(N_CORES):
        r, c = core // C_SHARDS, core % C_SHARDS
        out[r * T:(r + 1) * T, c * O:(c + 1) * O] = res.results[core]["out"]
    return out.reshape(4, 2048, O_FULL)
